# revision 1
# baseline (speedup 1.0000x reference)
"""Expert-parallel MoE FFN (ChronosMOEFeedForward) for 8 Trainium2 cores.

Strategy (sharding_hint: expert-parallel):
  - Router (softmax + top-2 over E=16 experts) computed on host in fp32 —
    top-k decisions must match the fp32 reference's ordering, and the router
    GEMM is ~0.1% of total FLOPs.
  - The 16 experts are sharded 2-per-core across 8 cores. Tokens routed to
    each expert are gathered on host (the "all-to-all dispatch"), padded to a
    fixed capacity C, and shipped transposed as [H, C] so the device GEMM
    chain needs no on-device transposes.
  - Per core the device computes, per expert e:
        gT = Wg[e].T @ XeT           [I, C]   (bf16 inputs, fp32 PSUM accum)
        uT = Wu[e].T @ XeT           [I, C]
        aT = silu(gT) * uT * w_bcast [I, C]   (combine weight broadcast on-chip)
        yT = Wd[e].T @ aT            [H, C]   (tokens stay on the free dim)
  - Host scatters each expert's y rows back to the owning tokens ("combine").
    A token's two expert contributions land in two disjoint slot arrays
    (top-1 slot, top-2 slot), so the combine is collision-free fancy
    indexing plus one add — no np.add.at.

Phase A runs as k-interleaved rounds over 8 concurrent PSUM banks so the PE
starts as soon as the first k-tile of weights lands, and the wg/wu halves
share a 2-slot pool so the next expert's weights stream in while the current
expert computes.

The dense reference formulation computes all 16 experts for every token;
routed top-2 computes only 2 — an 8x FLOP reduction, plus bf16 matmuls with
fp32 PSUM accumulation.
"""

import numpy as np
import ml_dtypes

import concourse.mybir as mybir
import concourse.tile as tile
from concourse import bacc
from concourse.bass_utils import run_bass_kernel_spmd

# Problem shapes (hardcoded per contract).
H = 2048        # hidden size
I = 1024        # moe intermediate size
E = 16          # num experts
TOPK = 2
B, S = 4, 1024
T = B * S       # 4096 tokens
N_CORES = 8
EPC = E // N_CORES  # experts per core = 2
# Per-slot token capacities: each core gets one "heavy" expert (slot 0) and
# one "light" expert (slot 1). The host assigns the 8 heaviest-loaded experts
# to slot 0. Mean load is 512 +- ~22, so the light half of the experts almost
# always fits in 512; the few tokens that overflow fall back to exact numpy.
CAPS = (640, 512)
C = CAPS[0]      # max capacity (DRAM params are padded to this)
NCHS = (320, 256)  # free-dim chunk per slot for the g/u matmuls (2 chunks)

BF16 = ml_dtypes.bfloat16

KT_H = H // 128  # 16 k-tiles over H
MT_I = I // 128  # 8 m-tiles over I
KT_I = I // 128  # 8 k-tiles over I

_CACHE = {}


def _build_nc(caps=CAPS, loop_r=None, internal=False):
    """Build the per-core Bass module (SPMD: all cores run this program).

    caps: per-slot token capacities. kernel() derives them from the actual
    routing (phase A streams tokens on the matmul free dim, so capacity is
    not 128-quantized there) and caches one compiled module per caps value.
    loop_r/internal are for the timing harness only: Internal DRAM I/O (no
    host transfers) with the body repeated loop_r times on-device.
    """
    import contextlib

    nc = bacc.Bacc(None, target_bir_lowering=False)
    f32 = mybir.dt.float32
    bf16 = mybir.dt.bfloat16

    if internal:
        xg = nc.dram_tensor("xg", [EPC, H, C], bf16)
        gww = nc.dram_tensor("gww", [EPC, H, I], bf16)
        uww = nc.dram_tensor("uww", [EPC, H, I], bf16)
        wdp = nc.dram_tensor("wdp", [EPC, I, H], bf16)
        wtv = nc.dram_tensor("wtv", [EPC, C], f32)
        y = nc.dram_tensor("y", [EPC, H, C], f32)
        done = nc.declare_dram_parameter("done", [1, 1], f32, isOutput=True)
    else:
        xg = nc.declare_dram_parameter("xg", [EPC, H, C], bf16, isOutput=False)
        gww = nc.declare_dram_parameter("gww", [EPC, H, I], bf16, isOutput=False)
        uww = nc.declare_dram_parameter("uww", [EPC, H, I], bf16, isOutput=False)
        wdp = nc.declare_dram_parameter("wdp", [EPC, I, H], bf16, isOutput=False)
        wtv = nc.declare_dram_parameter("wtv", [EPC, C], f32, isOutput=False)
        y = nc.declare_dram_parameter("y", [EPC, H, C], f32, isOutput=True)

    with tile.TileContext(nc) as tc:
        with (
            tc.tile_pool(name="wpool", bufs=2) as wpool,   # wg/wu halves share slots
            tc.tile_pool(name="xpool", bufs=1) as xpool,
            tc.tile_pool(name="wdpool", bufs=1) as wdpool,
            tc.tile_pool(name="apool", bufs=1) as apool,
            tc.tile_pool(name="small", bufs=2) as small,
            tc.tile_pool(name="yp", bufs=4) as yp,
            tc.tile_pool(name="ps", bufs=8, space="PSUM") as ps,
        ):
            const = small.tile([1, 128], f32, tag="ones")
            nc.any.memset(const[:], 1.0)

            loop_cm = (
                tc.For_i(0, loop_r, 1) if loop_r else contextlib.nullcontext()
            )
            with loop_cm:
                _emit_body(nc, tc, caps, xg, gww, uww, wdp, wtv, y, const,
                           wpool, xpool, wdpool, apool, small, yp, ps)

            if internal:
                dn = small.tile([1, 1], f32, tag="done")
                nc.any.memset(dn[:], 1.0)
                nc.sync.dma_start(out=done[:], in_=dn[:])

    nc.compile()
    return nc


def _emit_body(nc, tc, caps, xg, gww, uww, wdp, wtv, y, const,
               wpool, xpool, wdpool, apool, small, yp, ps):
    f32 = mybir.dt.float32
    bf16 = mybir.dt.bfloat16
    if True:
        if True:
            for e in range(EPC):
                Ce = caps[e]
                # one free-dim chunk if it fits a PSUM bank, else an even split
                NCH = Ce if Ce <= 512 else (Ce + 1) // 2
                # DMA issue order = need order: combine weights (tiny, feeds
                # the first PE instr group placed later), wg+xg k-tiles
                # interleaved (phase A ramp), then wu, then wd (phase B).
                wt_sb = small.tile([1, C], f32, tag="wt")
                nc.sync.dma_start(out=wt_sb[:, :Ce], in_=wtv[e][None, :Ce])
                gw_sb = wpool.tile([128, KT_H, I], bf16, tag="guw")
                xg_sb = xpool.tile([128, KT_H, C], bf16, tag="xg")
                for ko in range(KT_H):
                    nc.sync.dma_start(
                        out=gw_sb[:, ko, :], in_=gww[e, ko * 128 : (ko + 1) * 128, :]
                    )
                    nc.sync.dma_start(
                        out=xg_sb[:, ko, :Ce],
                        in_=xg[e, ko * 128 : (ko + 1) * 128, :Ce],
                    )
                uw_sb = wpool.tile([128, KT_H, I], bf16, tag="guw")
                for ko in range(KT_H):
                    nc.sync.dma_start(
                        out=uw_sb[:, ko, :], in_=uww[e, ko * 128 : (ko + 1) * 128, :]
                    )
                wd_sb = wdpool.tile([128, KT_I, H], bf16, tag="wd")
                for ko in range(KT_I):
                    nc.sync.dma_start(
                        out=wd_sb[:, ko, :], in_=wdp[e, ko * 128 : (ko + 1) * 128, :]
                    )

                sg_sb = apool.tile([128, MT_I, C], bf16, tag="sg")
                a_sb = apool.tile([128, MT_I, C], bf16, tag="a")
                wbc_sb = small.tile([128, C], bf16, tag="wbc")

                # broadcast combine weights across partitions via outer
                # product ones[128] x wt[C] -> wbc[128, C]; runs inside the
                # initial DMA ramp (wt is the first DMA issued) before any
                # g-group claims a psum bank
                for c0 in range(0, Ce, NCH):
                    w = min(NCH, Ce - c0)
                    pw = ps.tile([128, 512], f32, tag="ps")
                    nc.tensor.matmul(
                        pw[:, :w],
                        lhsT=const[:],
                        rhs=wt_sb[:, c0 : c0 + w],
                        start=True,
                        stop=True,
                    )
                    nc.vector.tensor_copy(wbc_sb[:, c0 : c0 + w], pw[:, :w])

                # ---- phase A: gT/uT in k-interleaved rounds of 8 psum groups
                for mat in range(2):  # 0: g (silu), 1: u (mul + weight)
                    w_sb = gw_sb if mat == 0 else uw_sb
                    for half in range(4):  # m-tile pairs; 4-group rounds so
                        # the other 4 psum banks' evictions overlap this round
                        groups = [
                            (m, c0, min(NCH, Ce - c0))
                            for m in range(half * 2, half * 2 + 2)
                            for c0 in range(0, Ce, NCH)
                        ]
                        psts = {}
                        for m, c0, w in groups:
                            psts[(m, c0)] = ps.tile([128, 512], f32, tag="ps", name=f"ps_{m}_{c0}")
                        for k in range(KT_H):
                            for m, c0, w in groups:
                                nc.tensor.matmul(
                                    psts[(m, c0)][:, :w],
                                    lhsT=w_sb[:, k, m * 128 : (m + 1) * 128],
                                    rhs=xg_sb[:, k, c0 : c0 + w],
                                    start=(k == 0),
                                    stop=(k == KT_H - 1),
                                )
                        for m, c0, w in groups:
                            pt = psts[(m, c0)][:, :w]
                            if mat == 0:
                                # silu(g) = g * sigmoid(g)
                                sig = small.tile([128, NCH], bf16, tag="sig")
                                nc.scalar.activation(
                                    sig[:, :w], pt, mybir.ActivationFunctionType.Sigmoid
                                )
                                nc.vector.tensor_mul(
                                    sg_sb[:, m, c0 : c0 + w], sig[:, :w], pt
                                )
                            else:
                                tmp = small.tile([128, NCH], bf16, tag="tmp")
                                nc.vector.tensor_mul(
                                    tmp[:, :w], sg_sb[:, m, c0 : c0 + w], pt
                                )
                                nc.vector.tensor_mul(
                                    a_sb[:, m, c0 : c0 + w],
                                    tmp[:, :w],
                                    wbc_sb[:, c0 : c0 + w],
                                )

                # ---- phase B: yT = Wd.T @ a   [H, Ce] — tokens stay on the
                # free dim, so only the exact Ce columns are streamed (no
                # 128-row quantization like the y = a.T @ Wd layout)
                bchunks = (
                    [(0, Ce)] if Ce <= 512 else [(0, NCH), (NCH, Ce - NCH)]
                )
                for pair in range(H // 256):  # m-tile pairs over H
                    bgroups = [
                        (m, c0, w)
                        for m in range(pair * 2, pair * 2 + 2)
                        for c0, w in bchunks
                    ]
                    bpsts = {}
                    for m, c0, w in bgroups:
                        bpsts[(m, c0)] = ps.tile(
                            [128, 512], f32, tag="ps", name=f"bps_{m}_{c0}"
                        )
                    for k in range(KT_I):
                        for m, c0, w in bgroups:
                            nc.tensor.matmul(
                                bpsts[(m, c0)][:, :w],
                                lhsT=wd_sb[:, k, m * 128 : (m + 1) * 128],
                                rhs=a_sb[:, k, c0 : c0 + w],
                                start=(k == 0),
                                stop=(k == KT_I - 1),
                            )
                    yts = {}
                    for m, c0, w in bgroups:
                        if c0 == 0:
                            yts[m] = yp.tile(
                                [128, C], f32, tag="ysb", name=f"yt_{m}"
                            )
                        nc.vector.tensor_copy(
                            yts[m][:, c0 : c0 + w], bpsts[(m, c0)][:, :w]
                        )
                    for m in range(pair * 2, pair * 2 + 2):
                        nc.sync.dma_start(
                            out=y[e, m * 128 : (m + 1) * 128, :Ce],
                            in_=yts[m][:, :Ce],
                        )


def _route(xf, gate_w):
    """Top-2 routing, mirroring the fp32 reference semantics exactly."""
    logits = xf @ gate_w.T.astype(np.float32)          # [T, E]
    logits -= logits.max(axis=-1, keepdims=True)
    scores = np.exp(logits)
    scores /= scores.sum(axis=-1, keepdims=True)
    i1 = scores.argmax(axis=-1)
    s1 = scores[np.arange(T), i1]
    masked = scores.copy()
    masked[np.arange(T), i1] = -np.inf
    i2 = masked.argmax(axis=-1)
    s2 = scores[np.arange(T), i2]
    denom = s1 + s2 + 1e-20
    return i1, s1 / denom, i2, s2 / denom


def _expert_np(xrows, wts, wg_e, wu_e, wd_e):
    """Exact fp32 fallback for capacity-overflow tokens (rare)."""
    g = xrows @ wg_e
    u = xrows @ wu_e
    a = (g / (1.0 + np.exp(-g))) * u * wts[:, None]
    return a @ wd_e


def _pack(xf, gate_w, wg, wu, wd):
    """Route + gather + pack per-core device inputs.

    Experts are assigned to (core, slot) by load: the 8 heaviest go to the
    C=640 slot 0, the 8 lightest to the C=512 slot 1. The assignment is pure
    host-side data placement — the SPMD program is identical on every core.
    """
    i1, w1, i2, w2 = _route(xf, gate_w)
    per_e = []
    for e in range(E):
        l1 = np.nonzero(i1 == e)[0]
        l2 = np.nonzero(i2 == e)[0]
        toks = np.concatenate([l1, l2])
        wts = np.concatenate([w1[l1], w2[l2]])
        ranks = np.concatenate(
            [np.zeros(len(l1), np.int8), np.ones(len(l2), np.int8)]
        )
        per_e.append((toks, ranks, wts))
    loads = [len(pe[0]) for pe in per_e]
    order = np.argsort([-n for n in loads], kind="stable")
    # exact capacities from this routing: slot 0 covers the heaviest expert
    # (up to the 640 DRAM padding), slot 1 stays at <=512 so its phase-B tile
    # count stays at 4; the rare overflow tokens go to the exact numpy path
    caps = (
        min(loads[order[0]], CAPS[0]),
        min(max(loads[order[N_CORES]], 128), CAPS[1]),
    )

    in_maps = []
    tok_lists = []
    for c in range(N_CORES):
        xgc = np.zeros((EPC, H, C), BF16)
        wtc = np.zeros((EPC, C), np.float32)
        core_toks = []
        experts = [int(order[c]), int(order[2 * N_CORES - 1 - c])]
        for j in range(EPC):
            e = experts[j]
            toks, ranks, wts = per_e[e]
            n_dev = min(len(toks), caps[j])
            xgc[j, :, :n_dev] = xf[toks[:n_dev]].T.astype(BF16)
            wtc[j, :n_dev] = wts[:n_dev]
            core_toks.append((toks, ranks, wts, n_dev, e))
        tok_lists.append(core_toks)
        in_maps.append(
            {
                "xg": xgc,
                "gww": wg[experts].astype(BF16),
                "uww": wu[experts].astype(BF16),
                "wdp": wd[experts].astype(BF16),
                "wtv": wtc,
            }
        )
    return in_maps, tok_lists, caps


def kernel(x, gate_w, wg, wu, wd):
    in_dtype = x.dtype
    xf = np.ascontiguousarray(x.reshape(T, H), dtype=np.float32)
    wg = np.asarray(wg, dtype=np.float32)
    wu = np.asarray(wu, dtype=np.float32)
    wd = np.asarray(wd, dtype=np.float32)

    in_maps, tok_lists, caps = _pack(xf, gate_w, wg, wu, wd)
    if caps not in _CACHE:
        _CACHE[caps] = _build_nc(caps)
    nc = _CACHE[caps]
    out1 = np.zeros((T, H), np.float32)
    out2 = np.zeros((T, H), np.float32)

    res = run_bass_kernel_spmd(nc, in_maps, core_ids=list(range(N_CORES)))
    _CACHE["last_in_maps"] = in_maps
    _CACHE["last_caps"] = caps
    _CACHE["nc"] = nc

    for c in range(N_CORES):
        yc = res.results[c]["y"].astype(np.float32)        # [EPC, H, C] (yT)
        for j in range(EPC):
            toks, ranks, wts, n_dev, e = tok_lists[c][j]
            yr = np.ascontiguousarray(yc[j, :, :n_dev].T)
            sel1 = ranks[:n_dev] == 0
            sel2 = ~sel1
            out1[toks[:n_dev][sel1]] = yr[sel1]
            out2[toks[:n_dev][sel2]] = yr[sel2]
            if len(toks) > n_dev:                          # capacity overflow
                extra = toks[n_dev:]
                yextra = _expert_np(xf[extra], wts[n_dev:], wg[e], wu[e], wd[e])
                r = ranks[n_dev:]
                out1[extra[r == 0]] = yextra[r == 0]
                out2[extra[r == 1]] = yextra[r == 1]

    out = (out1 + out2).reshape(B, S, H)
    return out.astype(in_dtype, copy=False)

